# revision 6
# baseline (speedup 1.0000x reference)
"""3x3 MedianBlur (zero-padded) for (8, 3, 1024, 1024) fp32 on 8 trn2 NeuronCores.

Strategy:
  - Pure data parallel: batch element i -> core i. Each core filters
    (3, 1024, 1024) = 12 MB.
  - Per core, per channel, 8 row-tiles of 128 rows ([128, 1024] fp32).
  - Vertical window alignment comes free from DMA: each tile is loaded 3x
    from HBM at row offsets -1/0/+1 (xm/x0/xp), so the vertical triple for
    row r sits at the same partition in the three tiles.
  - Separable exact median-of-9:
      stage V (per column): lo/me/hi of the vertical triple  (6 min/max ops)
      stage H (free-dim shifts): med9 = med3(max3(lo), med3(me), min3(hi))
                                                            (12 min/max ops)
    All 18 ops are fp32 tensor_tensor min/max on the vector engine (DVE),
    the only engine this toolchain allows 2-tensor elementwise ops on.
  - Zero padding: row pads via memset of halo partitions on edge tiles;
    column pads via 1-col zero borders on the stage-V outputs (sorted
    triple of the zero column is (0,0,0)).

The walrus build here accepts at most 1 inline sync wait per instruction
(2 on EventSemaphore); Tile emits more, so _legalize_waits() spills excess
waits onto same-engine NoOps placed immediately before the instruction.
"""

import sys

sys.path.insert(0, "/opt/trn_rl_repo")

import numpy as np

import concourse.bass as bass
import concourse.mybir as mybir
from concourse.bass_utils import run_bass_kernel_spmd
from concourse.tile import TileContext

C, H, W = 3, 1024, 1024
P = 128
NT = H // P  # 8 row tiles per channel
F32 = mybir.dt.float32
MIN = mybir.AluOpType.min
MAX = mybir.AluOpType.max


def _legalize_waits(nc):
    """Split sync_info.on_wait lists that exceed this walrus's per-instruction
    capacity (1; 2 for EventSemaphore) onto preceding same-engine NoOps."""
    for f in nc.m.functions:
        for bb in f.blocks:
            new_insts = []
            for ins in bb.instructions:
                si = ins.sync_info
                cap = 2 if ins.opcode == "EventSemaphore" else 1
                if si is not None and len(si.on_wait) > cap:
                    waits = list(si.on_wait)
                    extra, keep = waits[:-cap], waits[-cap:]
                    for w in extra:
                        nop = mybir.InstNoOp(
                            name=nc.get_next_instruction_name(),
                            ins=[],
                            outs=[],
                            engine=ins.engine,
                        )
                        nop.sync_info = mybir.SyncInfo(on_wait=[w], on_update=[])
                        new_insts.append(nop)
                    ins.sync_info = mybir.SyncInfo(
                        on_wait=keep, on_update=list(si.on_update)
                    )
                new_insts.append(ins)
            bb.instructions = new_insts


def build(bufs=2):
    nc = bass.Bass()
    x = nc.dram_tensor("x", [C, H, W], F32, kind="ExternalInput")
    y = nc.dram_tensor("y", [C, H, W], F32, kind="ExternalOutput")
    tt = nc.vector.tensor_tensor

    with TileContext(nc) as tc:
        with tc.tile_pool(name="main", bufs=bufs) as pool:
            for c in range(C):
                for t in range(NT):
                    r0 = t * P
                    x0 = pool.tile([P, W], F32, tag="x0")
                    xm = pool.tile([P, W], F32, tag="xm")
                    xp = pool.tile([P, W], F32, tag="xp")
                    nc.sync.dma_start(x0[:], x[c, r0 : r0 + P, :])
                    if t == 0:
                        nc.gpsimd.memset(xm[:], 0.0)
                        nc.sync.dma_start(xm[1:P, :], x[c, 0 : P - 1, :])
                    else:
                        nc.sync.dma_start(xm[:], x[c, r0 - 1 : r0 + P - 1, :])
                    if t == NT - 1:
                        nc.gpsimd.memset(xp[:], 0.0)
                        nc.sync.dma_start(xp[0 : P - 1, :], x[c, r0 + 1 : r0 + P, :])
                    else:
                        nc.sync.dma_start(xp[:], x[c, r0 + 1 : r0 + P + 1, :])

                    # stage V: sorted vertical triples, with 1-col zero pads
                    L = pool.tile([P, W + 2], F32, tag="L")
                    M = pool.tile([P, W + 2], F32, tag="M")
                    Hh = pool.tile([P, W + 2], F32, tag="Hh")
                    for z in (L, M, Hh):
                        nc.gpsimd.memset(z[:, 0:1], 0.0)
                        nc.gpsimd.memset(z[:, W + 1 : W + 2], 0.0)
                    u = pool.tile([P, W], F32, tag="u")
                    v = pool.tile([P, W], F32, tag="v")
                    t2 = pool.tile([P, W], F32, tag="t2")
                    tt(u[:], xm[:], x0[:], MIN)
                    tt(v[:], xm[:], x0[:], MAX)
                    tt(L[:, 1 : W + 1], u[:], xp[:], MIN)
                    tt(t2[:], v[:], xp[:], MIN)
                    tt(M[:, 1 : W + 1], u[:], t2[:], MAX)
                    tt(Hh[:, 1 : W + 1], v[:], xp[:], MAX)

                    # stage H: A = max3(L), Cc = min3(Hh), B = med3(M),
                    #          out = med3(A, B, Cc)
                    a = pool.tile([P, W + 1], F32, tag="a")
                    A = pool.tile([P, W], F32, tag="A")
                    cc = pool.tile([P, W + 1], F32, tag="cc")
                    Cc = pool.tile([P, W], F32, tag="Cc")
                    p = pool.tile([P, W + 1], F32, tag="p")
                    q = pool.tile([P, W + 1], F32, tag="q")
                    b1 = pool.tile([P, W], F32, tag="b1")
                    B = pool.tile([P, W], F32, tag="B")
                    m1 = pool.tile([P, W], F32, tag="m1")
                    m2 = pool.tile([P, W], F32, tag="m2")
                    m3 = pool.tile([P, W], F32, tag="m3")
                    o = pool.tile([P, W], F32, tag="o")
                    tt(a[:], L[:, 0 : W + 1], L[:, 1 : W + 2], MAX)
                    tt(A[:], a[:, 0:W], L[:, 2 : W + 2], MAX)
                    tt(cc[:], Hh[:, 0 : W + 1], Hh[:, 1 : W + 2], MIN)
                    tt(Cc[:], cc[:, 0:W], Hh[:, 2 : W + 2], MIN)
                    tt(p[:], M[:, 0 : W + 1], M[:, 1 : W + 2], MIN)
                    tt(q[:], M[:, 0 : W + 1], M[:, 1 : W + 2], MAX)
                    tt(b1[:], q[:, 0:W], M[:, 2 : W + 2], MIN)
                    tt(B[:], p[:, 0:W], b1[:], MAX)
                    tt(m1[:], A[:], B[:], MIN)
                    tt(m2[:], A[:], B[:], MAX)
                    tt(m3[:], m2[:], Cc[:], MIN)
                    tt(o[:], m1[:], m3[:], MAX)
                    nc.scalar.dma_start(y[c, r0 : r0 + P, :], o[:])

    _legalize_waits(nc)
    return nc


_NC = None


def kernel(input):
    global _NC
    if _NC is None:
        _NC = build()
    input = np.asarray(input, dtype=np.float32)
    in_maps = [{"x": np.ascontiguousarray(input[i])} for i in range(input.shape[0])]
    res = run_bass_kernel_spmd(_NC, in_maps, core_ids=list(range(len(in_maps))))
    return np.stack([r["y"] for r in res.results], axis=0)
